# revision 1
# baseline (speedup 1.0000x reference)
"""Trainium2 Bass kernel for AttentionDownsampler (nn_AttentionDownsampler_10264971837445).

Math (per batch b):
  patches[b, Y, X, p=(y,xi), c] = hr[b, c, 14Y+y, 14X+xi]
  logits[b, Y, X, p] = sum_c patches * w[c] + ab
  l2 = logits * mask[b, Y, X] * wkk[p] + bkk[p]
  attn = softmax_p(l2)
  out[b, c, Y, X] = sum_p patches[..., p, c] * attn[p]

Sharding: 8 cores = 4 batches x 2 halves of the H(=Y) axis. The per-core
shard is laid out patch-contiguous on the host: [384, 8 rows, 16 X, 196 px]
(pure indexing; the DMA then streams 12.5 KB/partition contiguous runs and
every on-chip operand is a [128, 196] single-free-dim AP, which the DVE
TENSOR_TENSOR_REDUCE instruction requires — its ISA struct is S2S2D2).

Per-core kernel (8 patch-row iterations):
  - DMA 3 c-chunk tiles [128, 16, 196]
  - PE scoring: 48 matmuls w/ one-hot-expanded weights -> PSUM logits [16, 196]
  - softmax on [16, 196] (DVE affine/max, ACT exp w/ accum sum, DVE recip+scale)
  - attn [16,196] -> DRAM scratch row (DMA), then zero-stride broadcast-load
    back as attnB [128, 16, 196]
  - DVE affine_mul_reduce per (chunk, X): sum_p data*attn -> out col [128, 1]
  - outputs accumulate in SBUF [128, 8, 16] per chunk; one DMA per chunk at end
"""

import sys

for _p in ("/opt/trn_rl_repo", "/root/.axon_site/_ro/trn_rl_repo"):
    if _p not in sys.path:
        sys.path.append(_p)

import numpy as np

import concourse.bacc as bacc
import concourse.mybir as mybir
import concourse.tile as tile
from concourse.bass_utils import run_bass_kernel_spmd

K = 14          # patch size
C = 384         # channels
CCH = 128       # channel chunk (partitions)
NCH = C // CCH  # 3 chunks
NX = 16         # patches across W
P = K * K       # 196 pixels per patch
NCORES = 8

FP32 = mybir.dt.float32


def build_nc(nrow=8):
    """Build the SPMD Bass program (identical on all cores)."""
    nc = bacc.Bacc("TRN2", target_bir_lowering=False, debug=False,
                   num_devices=NCORES)

    # patch-grouped shard: [c, row, X, p]
    hr = nc.dram_tensor("hr", [C, nrow, NX, P], FP32, kind="ExternalInput")
    # one-hot expanded scorer weights: woh[c, X, m] = w[c] if m == X else 0
    woh = nc.dram_tensor("woh", [C, NX, NX], FP32, kind="ExternalInput")
    ones1 = nc.dram_tensor("ones1", [1, CCH], FP32, kind="ExternalInput")
    maskf = nc.dram_tensor("maskf", [NX, nrow], FP32, kind="ExternalInput")
    wkkb = nc.dram_tensor("wkkb", [NX, P], FP32, kind="ExternalInput")
    bkkb = nc.dram_tensor("bkkb", [NX, P], FP32, kind="ExternalInput")
    ab16 = nc.dram_tensor("ab16", [NX, 1], FP32, kind="ExternalInput")
    out = nc.dram_tensor("out", [C, nrow, NX], FP32, kind="ExternalOutput")
    attn_dram = nc.dram_tensor("attn_scratch", [nrow, NX, P], FP32)

    with tile.TileContext(nc) as tc:
        _emit(tc, nc, nrow, hr, woh, ones1, maskf, wkkb, bkkb, ab16, out, attn_dram)
    nc.finalize()
    return nc


def _emit(tc, nc, nrow, hr, woh, ones1, maskf, wkkb, bkkb, ab16, out, attn_dram):
    import contextlib
    ctx = contextlib.ExitStack()
    with ctx:
        singles = ctx.enter_context(tc.tile_pool(name="singles", bufs=1))
        data_pool = ctx.enter_context(tc.tile_pool(name="data", bufs=9))
        small = ctx.enter_context(tc.tile_pool(name="small", bufs=4))
        attnb_pool = ctx.enter_context(tc.tile_pool(name="attnb", bufs=3))
        outp = ctx.enter_context(tc.tile_pool(name="outp", bufs=6))
        dummy_pool = ctx.enter_context(tc.tile_pool(name="dummy", bufs=1))
        psum_lg = ctx.enter_context(
            tc.tile_pool(name="psum_lg", bufs=2, space="PSUM"))
        psum_bc = ctx.enter_context(
            tc.tile_pool(name="psum_bc", bufs=2, space="PSUM"))

        # ---- constants (loaded once) ----
        woh_sb = singles.tile([CCH, NCH, NX, NX], FP32)
        for k in range(NCH):
            nc.sync.dma_start(out=woh_sb[:, k, :, :],
                              in_=woh[k * CCH:(k + 1) * CCH, :, :])
        ones_sb = singles.tile([1, CCH], FP32)
        nc.sync.dma_start(out=ones_sb, in_=ones1[:, :])
        maskf_sb = singles.tile([NX, nrow], FP32)
        nc.sync.dma_start(out=maskf_sb, in_=maskf[:, :])
        wkkb_sb = singles.tile([NX, P], FP32)
        nc.sync.dma_start(out=wkkb_sb, in_=wkkb[:, :])
        bkkb_sb = singles.tile([NX, P], FP32)
        nc.sync.dma_start(out=bkkb_sb, in_=bkkb[:, :])
        ab_sb = singles.tile([NX, 1], FP32)
        nc.sync.dma_start(out=ab_sb, in_=ab16[:, :])

        dummy = dummy_pool.tile([CCH, 1], FP32, tag="dummy")
        osb_big = [singles.tile([CCH, nrow, NX], FP32, name=f"osbbig{k}",
                               tag=f"osb{k}") for k in range(NCH)]

        for r in range(nrow):
            # ---- load data tiles ----
            dk = []
            for k in range(NCH):
                t = data_pool.tile([CCH, NX, P], FP32, tag="data")
                nc.sync.dma_start(out=t, in_=hr[k * CCH:(k + 1) * CCH, r, :, :])
                dk.append(t)

            # ---- scoring: one accumulation group of 48 matmuls ----
            lg = psum_lg.tile([NX, P], FP32, tag="lg")
            for X in range(NX):
                for k in range(NCH):
                    nc.tensor.matmul(
                        lg[:, :],
                        woh_sb[:, k, X, :],
                        dk[k][:, X, :],
                        start=(X == 0 and k == 0),
                        stop=(X == NX - 1 and k == NCH - 1),
                    )

            # ---- softmax over p (per X partition) ----
            mw = small.tile([NX, P], FP32, tag="mw")
            nc.vector.tensor_scalar_mul(mw, wkkb_sb, maskf_sb[:, r:r + 1])
            l2 = small.tile([NX, P], FP32, tag="l2")
            nc.scalar.add(l2, lg[:, :], ab_sb[:, 0:1])  # PSUM -> SBUF on ACT
            nc.vector.tensor_mul(l2, l2, mw)
            nc.vector.tensor_add(l2, l2, bkkb_sb)
            negmax = small.tile([NX, 1], FP32, tag="negmax")
            nc.vector.tensor_reduce(negmax, l2, axis=mybir.AxisListType.X,
                                    op=mybir.AluOpType.max, negate=True)
            esum = small.tile([NX, 1], FP32, tag="esum")
            ex = small.tile([NX, P], FP32, tag="ex")
            nc.scalar.activation(ex, l2, mybir.ActivationFunctionType.Exp,
                                 bias=negmax[:, 0:1], scale=1.0,
                                 accum_out=esum[:, 0:1])
            rcp = small.tile([NX, 1], FP32, tag="rcp")
            nc.vector.reciprocal(rcp, esum)
            attn = small.tile([NX, P], FP32, tag="attn")
            nc.vector.tensor_scalar_mul(attn, ex, rcp[:, 0:1])

            # ---- gather attn to a single row, then broadcast to 128 parts ----
            nc.gpsimd.dma_start(out=attn_dram[r, :, :], in_=attn[:, :])
            attnB = attnb_pool.tile([CCH, NX, P], FP32, tag="attnB")
            import concourse.bass as bass_mod
            _src = attn_dram[r, :, :]
            _bsrc = bass_mod.AP(tensor=_src.tensor, offset=_src.offset,
                                ap=[[0, CCH], *_src.ap])
            nc.gpsimd.dma_start(out=attnB, in_=_bsrc)

            # ---- pass B: fused multiply + reduce over p, per (chunk, X) ----
            for k in range(NCH):
                for X in range(NX):
                    nc.vector.affine_mul_reduce(
                        out=dummy.broadcast_to((CCH, P)),
                        accum_out=osb_big[k][:, r, X:X + 1],
                        in0=dk[k][:, X, :],
                        in1=attnB[:, X, :],
                        scale=1.0,
                        bias=0.0,
                    )

        for k in range(NCH):
            nc.sync.dma_start(out=out[k * CCH:(k + 1) * CCH, :, :],
                              in_=osb_big[k])


_NC_CACHE = {}


def _get_nc(nrow=8):
    if nrow not in _NC_CACHE:
        _NC_CACHE[nrow] = build_nc(nrow)
    return _NC_CACHE[nrow]


def regroup_shard(hr_slice):
    """[384, 112, 224] -> patch-grouped [384, 8, 16, 196] (pure indexing)."""
    c, h, w = hr_slice.shape
    g = hr_slice.reshape(c, h // K, K, w // K, K).transpose(0, 1, 3, 2, 4)
    return np.ascontiguousarray(g.reshape(c, h // K, w // K, P), np.float32)


def make_in_maps(hr_feats, guidance, attn_w, attn_b, w_kk, b_kk, dropout_mask,
                 nrow=8):
    b = hr_feats.shape[0]
    w = np.asarray(attn_w, np.float32)[0]                      # [384]
    woh = np.zeros((C, NX, NX), np.float32)
    woh[:, np.arange(NX), np.arange(NX)] = w[:, None]          # [c, X, m]
    ones1 = np.ones((1, CCH), np.float32)
    wkk_flat = np.asarray(w_kk, np.float32).reshape(-1)        # [196]
    wkkb = np.tile(wkk_flat[None, :], (NX, 1))
    bkkb = np.tile(np.asarray(b_kk, np.float32).reshape(-1)[None, :], (NX, 1))
    ab16 = np.full((NX, 1), np.float32(np.asarray(attn_b)[0]), np.float32)
    mask = np.asarray(dropout_mask).astype(np.float32)[..., 0]  # [b, H, W]

    in_maps = []
    for core in range(NCORES):
        bi, half = divmod(core, 2)
        bi = bi % b
        hrg = regroup_shard(
            np.asarray(hr_feats[bi, :, 112 * half:112 * half + K * nrow, :],
                       np.float32))
        maskf = np.ascontiguousarray(
            mask[bi, 8 * half:8 * half + nrow, :].T)           # [X, r]
        in_maps.append({
            "hr": hrg, "woh": woh, "ones1": ones1, "maskf": maskf,
            "wkkb": wkkb, "bkkb": bkkb, "ab16": ab16,
        })
    return in_maps


def kernel(hr_feats, guidance, attn_w, attn_b, w_kk, b_kk, dropout_mask,
           trace=False):
    hr_feats = np.asarray(hr_feats, np.float32)
    b, c, h, wimg = hr_feats.shape
    H = h // K
    nc = _get_nc(8)
    in_maps = make_in_maps(hr_feats, guidance, attn_w, attn_b, w_kk, b_kk,
                           dropout_mask)
    res = run_bass_kernel_spmd(nc, in_maps, core_ids=list(range(NCORES)),
                               trace=trace)
    full = np.empty((b, C, H, NX), np.float32)
    for core in range(NCORES):
        bi, half = divmod(core, 2)
        full[bi, :, 8 * half:8 * half + 8, :] = res.results[core]["out"]
    if trace:
        return full, res
    return full

